# revision 10
# baseline (speedup 1.0000x reference)
"""Trainium2 Bass kernel for nn_DenseExpert (soft-gated mixture of dense experts).

Math:  out[b,u] = sum_e gate[b,e] * (x[b,:] @ alpha[e]) [u] + (gate @ beta)[b,u]

Hybrid strategy (pure data parallel over batch, 8 cores; BLOC=8192/core).
The bias term (gate @ beta, 1.5% of FLOPs) is added on the host.

Per 512-row group (16 groups of 4 tiles x 128 rows), the 8 experts split
across two scaled-transpose pipelines that run on different hardware:

  Experts 0-3 ("xbar path"):
    DVE gate-scales x into 4 expert copies (tensor_tensor in 2x_1p mode
    via a host-duplicated gate pair layout), then ONE xbar-DMA transpose
    per group turns [q, (e,t), i] into [i, (e,t), q] off the compute
    engines. (The xbar is a serial device: keep all transposes on one
    ring - concurrent transposes from two rings corrupt data.)

  Experts 4-7 ("diag path", PE does the scale+transpose):
    The host precomputes dstk[q, T, d, c] = gate[128T+q, 4+d] * [c==q%64].
    Two matmuls per tile (x block-stationary, dstk moving) produce
    yT[i, (d,c)] = gate * x transposed directly in PSUM; ACT gathers
    PSUM->SBUF with f32->f16 cast.

  Mains: 8 accumulating matmuls per group (N=512):
    oT[u, (t,q)] += alpha_e[i,u].T @ scaled-xT_e.
  DVE evicts oT (f32->f16); out DMA per group on the scalar ring.

Host: x/gate cast to f16, dstk/g2 packing, alpha pre-transposed to
[i, e, u]; output [u, b] f16 transposed back, cast to f32, bias added.
"""

import dataclasses
from contextlib import ExitStack

import numpy as np

import concourse.bacc as bacc
import concourse.tile as tile
import concourse.mybir as mybir
from concourse.bass_utils import run_bass_kernel_spmd

F32 = mybir.dt.float32
F16 = mybir.dt.float16

B, E, I, U = 65536, 8, 128, 128
NCORES = 8
BLOC = B // NCORES        # 8192 rows per core
NT = BLOC // 128          # 64 tiles of 128 rows
GT = 4                    # tiles per group
NG = NT // GT             # 16 groups
GROUP_COLS = GT * 128     # 512 columns of out.T per group
KX = 4                    # experts on the xbar path (0..KX-1)
MD = E - KX               # experts on the PE-diag path (KX..E-1)
KB = 128                  # diag block size (full tile; c == q)


def _build():
    nc = bacc.Bacc("TRN2", target_bir_lowering=False, debug=False)

    x16 = nc.dram_tensor("x16", [BLOC, I], F16, kind="ExternalInput").ap()
    # g2[q, T, e, j] = gate[128T+q, e] duplicated over j in {0,1}
    g2 = nc.dram_tensor("g2", [128, NT, KX, 2], F16, kind="ExternalInput").ap()
    # al[i, e, u] = alpha[e, i, u]
    al = nc.dram_tensor("al", [128, E, U], F16, kind="ExternalInput").ap()
    # dstk[q, T, d, c] = gate[128T+q, KX+d] * (c == q)
    dstk = nc.dram_tensor("dstk", [128, NT, MD, KB], F16, kind="ExternalInput").ap()
    # out.T, columns ordered (T, q): col = 128T + q = batch row
    outT = nc.dram_tensor("outT", [U, BLOC], F16, kind="ExternalOutput").ap()

    with tile.TileContext(nc) as tc, ExitStack() as ctx:
        const = ctx.enter_context(tc.tile_pool(name="const", bufs=1))
        xep = ctx.enter_context(tc.tile_pool(name="xep", bufs=4))
        xtp = ctx.enter_context(tc.tile_pool(name="xtp", bufs=4))
        ytp = ctx.enter_context(tc.tile_pool(name="ytp", bufs=4))
        osp = ctx.enter_context(tc.tile_pool(name="osp", bufs=2))
        psy = ctx.enter_context(tc.tile_pool(name="psy", bufs=2, space="PSUM"))
        pso = ctx.enter_context(tc.tile_pool(name="pso", bufs=2, space="PSUM"))

        x_all = const.tile([128, NT, I], F16, tag="x")
        for q4 in range(4):
            nc.sync.dma_start(
                x_all[:, q4 * 16:(q4 + 1) * 16, :],
                x16[q4 * 2048:(q4 + 1) * 2048, :].rearrange(
                    "(T p) i -> p T i", p=128
                ),
            )
        g_sb = const.tile([128, NT, KX, 2], F16, tag="g")
        nc.sync.dma_start(g_sb[:], g2)
        al_sb = const.tile([128, E, U], F16, tag="al")
        nc.sync.dma_start(al_sb[:], al)
        dstk_sb = const.tile([128, NT, MD, KB], F16, tag="dstk")
        for q4 in range(4):
            nc.scalar.dma_start(
                dstk_sb[:, q4 * 16:(q4 + 1) * 16, :, :],
                dstk[:, q4 * 16:(q4 + 1) * 16, :, :],
            )

        g_pitch = NT * KX * 2
        for G in range(NG):
            # --- xbar path: gate-scale experts 0..KX-1, one DVE op per tile
            xe = xep.tile([128, KX, GT, I], F16, tag="xe")
            for t in range(GT):
                T = G * GT + t
                xe_v = dataclasses.replace(
                    xe[:],
                    ap=[[KX * GT * I, 128], [GT * I, KX], [2, 64], [1, 2]],
                    offset=t * I,
                )
                x_v = dataclasses.replace(
                    x_all[:],
                    ap=[[NT * I, 128], [0, KX], [2, 64], [1, 2]],
                    offset=T * I,
                )
                g_v = dataclasses.replace(
                    g_sb[:],
                    ap=[[g_pitch, 128], [2, KX], [0, 64], [1, 2]],
                    offset=T * KX * 2,
                )
                nc.vector.tensor_tensor(xe_v, x_v, g_v, op=mybir.AluOpType.mult)

            # xbar transpose (single ring!): xeT[i, (e,t), q]
            xeT = xtp.tile([128, KX * GT, 128], F16, tag="xeT")
            xe2d = dataclasses.replace(
                xe[:], ap=[[KX * GT * I, 128], [1, KX * GT * I]], offset=0
            )
            nc.sync.dma_start_transpose(xeT[:], xe2d)

            # --- diag path: experts KX..E-1 via PE (KB=128: one matmul per
            # tile, full-partition stationary x and moving dstk; column c==q)
            # yT_sb layout [t, d, c]; psum per tile-pair [t2, (d c)]
            yT_sb = ytp.tile([128, GT, MD, KB], F16, tag="yT")
            for tp in range(2):
                yps = psy.tile([128, 2, MD * KB], F32, tag="yps")
                for t2 in range(2):
                    T = G * GT + 2 * tp + t2
                    nc.tensor.matmul(
                        yps[:, t2, :],
                        x_all[:, T, :],
                        dstk_sb[:, T, :, :],
                        start=True,
                        stop=True,
                    )
                # gather PSUM -> SBUF (ACT): flat contiguous copy
                gdst = dataclasses.replace(
                    yT_sb[:],
                    ap=[[GT * MD * KB, 128], [1, 2 * MD * KB]],
                    offset=tp * 2 * MD * KB,
                )
                gsrc = dataclasses.replace(
                    yps[:],
                    ap=[[2 * MD * KB, 128], [1, 2 * MD * KB]],
                    offset=0,
                )
                nc.scalar.copy(gdst, gsrc)

            # --- mains: oT[u, (t, q)] accumulated over all 8 experts
            oT = pso.tile([128, GT, 128], F32, tag="oT")
            for d in range(MD):
                mv = dataclasses.replace(
                    yT_sb[:],
                    ap=[[GT * MD * KB, 128], [MD * KB, GT], [1, KB]],
                    offset=d * KB,
                )
                nc.tensor.matmul(
                    oT[:],
                    al_sb[:, KX + d, :],
                    mv,
                    start=(d == 0),
                    stop=False,
                )
            for e in range(KX):
                nc.tensor.matmul(
                    oT[:],
                    al_sb[:, e, :],
                    xeT[:, GT * e:GT * (e + 1), :],
                    start=False,
                    stop=(e == KX - 1),
                )

            # --- evict (DVE) + out DMA (scalar ring)
            o_sb = osp.tile([128, GT, 128], F16, tag="o")
            nc.vector.tensor_copy(o_sb[:], oT[:])
            nc.scalar.dma_start(
                outT[:, G * GROUP_COLS:(G + 1) * GROUP_COLS],
                o_sb[:].rearrange("u t q -> u (t q)"),
            )

    nc.compile()
    return nc


_NC_CACHE = None


def make_in_maps(x, gate_perc, alpha, beta=None):
    x16 = np.asarray(x, dtype=np.float16)
    g16 = np.asarray(gate_perc, dtype=np.float16)
    al16 = np.ascontiguousarray(
        np.asarray(alpha, dtype=np.float16).transpose(1, 0, 2)
    )
    in_maps = []
    for c in range(NCORES):
        sl = slice(c * BLOC, (c + 1) * BLOC)
        gc = g16[sl].reshape(NT, 128, E).transpose(1, 0, 2)  # [q, T, e]
        # g2[q, T, e, j] for xbar experts
        g2 = np.ascontiguousarray(
            np.broadcast_to(
                gc[:, :, :KX, None], (128, NT, KX, 2)
            ).astype(np.float16)
        )
        # dstk[q, T, d, c] = gc[q, T, KX+d] * (c == q)
        dstk = np.zeros((128, NT, MD, KB), np.float16)
        dstk[np.arange(128)[:, None, None],
             np.arange(NT)[None, :, None],
             np.arange(MD)[None, None, :],
             np.arange(128)[:, None, None]] = gc[:, :, KX:]
        in_maps.append(
            {
                "x16": np.ascontiguousarray(x16[sl]),
                "g2": g2,
                "al": al16,
                "dstk": dstk,
            }
        )
    return in_maps


def assemble(results, gate_perc, beta):
    # per-core outT is [U, BLOC] f16 with column b = batch row within core
    full_T = np.concatenate([results[c]["outT"] for c in range(NCORES)], axis=1)
    out = np.ascontiguousarray(full_T.T).astype(np.float32)
    out += np.asarray(gate_perc, dtype=np.float32) @ np.asarray(beta, dtype=np.float32)
    return out


def kernel(x, gate_perc, alpha, beta):
    global _NC_CACHE
    if _NC_CACHE is None:
        _NC_CACHE = _build()
    nc = _NC_CACHE

    in_maps = make_in_maps(x, gate_perc, alpha)
    res = run_bass_kernel_spmd(nc, in_maps, list(range(NCORES))).results
    return assemble(res, gate_perc, beta)


if __name__ == "__main__":
    rng = np.random.default_rng(0)
    x = rng.standard_normal((B, I)).astype(np.float32)
    g = rng.random((B, E)).astype(np.float32)
    g /= g.sum(-1, keepdims=True)
    al = (rng.standard_normal((E, I, U)) * 0.05).astype(np.float32)
    be = (rng.standard_normal((E, U)) * 0.05).astype(np.float32)
    got = kernel(x, g, al, be)
    ref = np.einsum("bi,eio->beo", x, al, optimize=True)
    ref = np.einsum("beo,be->bo", ref, g) + g @ be
    err = np.abs(got - ref)
    print("max abs err", err.max(), "rel", err.max() / np.abs(ref).max())


# revision 11
# speedup vs baseline: 1.1386x; 1.1386x over previous
"""Trainium2 Bass kernel for nn_DenseExpert (soft-gated mixture of dense experts).

Math:  out[b,u] = sum_e gate[b,e] * (x[b,:] @ alpha[e]) [u] + (gate @ beta)[b,u]

Strategy (pure data parallel over batch, 8 cores). Per 512-row chunk per core:
  1. DMA x/gate chunk (batch-major); cast to fp16 (11-bit mantissa, close to
     TF32 precision; PE streams fp16 at 1 cycle/row; PSUM accumulation fp32).
  2. Build K=64 block-diagonal gate matrices: dstack[p, e, c] =
     gate[p,e]*[c == p%64], one fp16 tensor_tensor per 128-row tile
     (ident-pattern * gate broadcast, FD=512) on DVE.
  3. y_e.T tiles via PE matmuls: for each 64-row block l,
     yT[i, (e, c)] = x[64l:64l+64, :].T @ dstack[64l:64l+64]   (N=512).
     This replaces both a scale stage and per-expert PE transposes.
  4. PSUM->SBUF copies gather yT into [i, e, b] fp16 layout (DVE/ACT split).
  5. PE matmuls accumulate out.T[u,b] = sum_e alpha_e.T @ y_e.T  plus the
     bias as one K=8 matmul beta.T @ gate.T (gate.T via 4 tiny identity
     matmuls).
  6. out.T (fp32) copied to SBUF and DMA'd to DRAM in [U, B] layout; the
     host does the final cheap transpose when assembling the full result.
"""

import dataclasses
from contextlib import ExitStack

import numpy as np

import concourse.bacc as bacc
import concourse.tile as tile
import concourse.mybir as mybir
from concourse.bass_utils import run_bass_kernel_spmd

F32 = mybir.dt.float32
F16 = mybir.dt.float16

B, E, I, U = 65536, 8, 128, 128
NCORES = 8
BLOC = B // NCORES        # 8192 batch rows per core
CHUNK = 512               # batch rows per pipeline chunk
NCHUNK = BLOC // CHUNK    # 16
TPC = CHUNK // 128        # 128-row tiles per chunk
KB = 64                   # contraction block for the diag trick


def _build():
    nc = bacc.Bacc("TRN2", target_bir_lowering=False, debug=False)

    x = nc.dram_tensor("x", [BLOC, I], F32, kind="ExternalInput").ap()
    gate = nc.dram_tensor("gate", [BLOC, E], F32, kind="ExternalInput").ap()
    alpha = nc.dram_tensor("alpha", [E, I, U], F32, kind="ExternalInput").ap()
    beta = nc.dram_tensor("beta", [E, U], F32, kind="ExternalInput").ap()
    ident = nc.dram_tensor("ident", [128, 128], F16, kind="ExternalInput").ap()
    idrep = nc.dram_tensor("idrep", [128, E, KB], F16, kind="ExternalInput").ap()
    # output stays feature-major on HW; host transposes when assembling
    outT = nc.dram_tensor("outT", [U, BLOC], F32, kind="ExternalOutput").ap()

    with tile.TileContext(nc) as tc, ExitStack() as ctx:
        const = ctx.enter_context(tc.tile_pool(name="const", bufs=1))
        xp = ctx.enter_context(tc.tile_pool(name="xp", bufs=6))
        dgp = ctx.enter_context(tc.tile_pool(name="dgp", bufs=8))
        ytp = ctx.enter_context(tc.tile_pool(name="ytp", bufs=4))
        op = ctx.enter_context(tc.tile_pool(name="op", bufs=3))
        gp = ctx.enter_context(tc.tile_pool(name="gp", bufs=3))
        ps_yt = ctx.enter_context(tc.tile_pool(name="ps_yt", bufs=3, space="PSUM"))
        ps_ot = ctx.enter_context(tc.tile_pool(name="ps_ot", bufs=1, space="PSUM"))
        ps_gt = ctx.enter_context(tc.tile_pool(name="ps_gt", bufs=1, space="PSUM"))

        # --- constants (cast alpha/beta to fp16 on chip) ---
        alpha_sb = const.tile([128, E, U], F32, tag="alpha")
        nc.sync.dma_start(alpha_sb[:], alpha.rearrange("e i u -> i e u"))
        alpha_h = const.tile([128, E, U], F16, tag="alphah")
        nc.vector.tensor_copy(alpha_h[:], alpha_sb[:])

        beta_sb = const.tile([8, U], F32, tag="beta")
        nc.sync.dma_start(beta_sb[:], beta)
        beta_h = const.tile([8, U], F16, tag="betah")
        nc.vector.tensor_copy(beta_h[:], beta_sb[:])

        ident_h = const.tile([128, 128], F16, tag="identh")
        nc.sync.dma_start(ident_h[:], ident)
        idrep_h = const.tile([128, E, KB], F16, tag="idreph")
        nc.sync.dma_start(idrep_h[:], idrep)

        def emit_front(c):
            row0 = c * CHUNK
            g_sb = xp.tile([128, TPC, E], F32, tag="g")
            nc.sync.dma_start(
                g_sb[:],
                gate[row0 : row0 + CHUNK, :].rearrange("(t p) e -> p t e", p=128),
            )
            # x: SWDGE DMA with fused fp32->fp16 cast (issued from GpSimd,
            # which is otherwise idle)
            x_h = xp.tile([128, TPC, I], F16, tag="xh")
            nc.gpsimd.dma_start(
                x_h[:], x[row0 : row0 + CHUNK, :].rearrange("(t p) i -> p t i", p=128)
            )
            g_h = xp.tile([128, TPC, E], F16, tag="gh")
            nc.vector.tensor_copy(g_h[:], g_sb[:])

            # gate.T for the bias matmul, via tiny identity matmuls
            gT_ps = ps_gt.tile([E, TPC, 128], F32, tag="gTps")
            for t in range(TPC):
                nc.tensor.matmul(
                    gT_ps[:, t, :], g_h[:, t, :], ident_h[:], start=True, stop=True
                )
            gT_h = gp.tile([E, TPC, 128], F16, tag="gTh")
            nc.vector.tensor_copy(gT_h[:], gT_ps[:])

            # per 128-row tile: diag build (DVE) + yT matmuls + gather copy
            yT_all = ytp.tile([128, E, TPC, 128], F16, tag="yT")
            for t in range(TPC):
                gview = dataclasses.replace(
                    g_h[:],
                    ap=[[TPC * E, 128], [1, E], [0, KB]],
                    offset=t * E,
                )
                diag = dgp.tile([128, E, KB], F16, tag="diag")
                nc.vector.tensor_tensor(
                    diag[:], idrep_h[:], gview, op=mybir.AluOpType.mult
                )
                yT_ps = ps_yt.tile([128, 2, E, KB], F32, tag="yTps")
                for l in range(2):
                    nc.tensor.matmul(
                        yT_ps[:, l, :, :],
                        x_h[l * KB : (l + 1) * KB, t, :],
                        diag[l * KB : (l + 1) * KB, :, :],
                        start=True,
                        stop=True,
                    )
                dst = dataclasses.replace(
                    yT_all[:],
                    ap=[[E * TPC * 128, 128], [KB, 2], [TPC * 128, E], [1, KB]],
                    offset=t * 128,
                )
                if t == 3:
                    # split the last tile's gather across DVE and ACT
                    dst0 = dataclasses.replace(
                        yT_all[:],
                        ap=[[E * TPC * 128, 128], [TPC * 128, E], [1, KB]],
                        offset=t * 128,
                    )
                    dst1 = dataclasses.replace(
                        yT_all[:],
                        ap=[[E * TPC * 128, 128], [TPC * 128, E], [1, KB]],
                        offset=t * 128 + KB,
                    )
                    nc.vector.tensor_copy(dst0, yT_ps[:, 0, :, :])
                    nc.scalar.copy(dst1, yT_ps[:, 1, :, :])
                else:
                    nc.scalar.copy(dst, yT_ps[:])
            return yT_all, gT_h

        def emit_back(c, yT_all, gT_h):
            row0 = c * CHUNK
            oT_ps = ps_ot.tile([128, CHUNK], F32, tag="oTps")
            for e in range(E):
                nc.tensor.matmul(
                    oT_ps[:],
                    alpha_h[:, e, :],
                    yT_all[:, e, :, :],
                    start=(e == 0),
                    stop=False,
                )
            nc.tensor.matmul(oT_ps[:], beta_h[:], gT_h[:], start=False, stop=True)

            oT_sb = op.tile([128, CHUNK], F32, tag="oT")
            nc.vector.tensor_copy(oT_sb[:, : CHUNK // 2], oT_ps[:, : CHUNK // 2])
            nc.scalar.copy(oT_sb[:, CHUNK // 2 :], oT_ps[:, CHUNK // 2 :])
            nc.sync.dma_start(outT[:, row0 : row0 + CHUNK], oT_sb[:])

        pending = None
        for c in range(NCHUNK):
            front = emit_front(c)
            if pending is not None:
                emit_back(c - 1, *pending)
            pending = front
        emit_back(NCHUNK - 1, *pending)

    nc.compile()
    return nc


_NC_CACHE = None


def _make_idrep():
    idrep = np.zeros((128, E, KB), np.float16)
    for p in range(128):
        idrep[p, :, p % KB] = 1.0
    return idrep


def make_in_maps(x, gate_perc, alpha, beta):
    x = np.ascontiguousarray(np.asarray(x, dtype=np.float32))
    gate_perc = np.ascontiguousarray(np.asarray(gate_perc, dtype=np.float32))
    alpha = np.ascontiguousarray(np.asarray(alpha, dtype=np.float32))
    beta = np.ascontiguousarray(np.asarray(beta, dtype=np.float32))
    ident = np.eye(128, dtype=np.float16)
    idrep = _make_idrep()
    in_maps = []
    for c in range(NCORES):
        sl = slice(c * BLOC, (c + 1) * BLOC)
        in_maps.append(
            {
                "x": x[sl],
                "gate": gate_perc[sl],
                "alpha": alpha,
                "beta": beta,
                "ident": ident,
                "idrep": idrep,
            }
        )
    return in_maps


def assemble(results, gate_perc=None, beta=None):
    # per-core outputs are [U, BLOC] f32 (bias already applied on device)
    full_T = np.concatenate([results[c]["outT"] for c in range(NCORES)], axis=1)
    return np.ascontiguousarray(full_T.T)


def kernel(x, gate_perc, alpha, beta):
    global _NC_CACHE
    if _NC_CACHE is None:
        _NC_CACHE = _build()
    nc = _NC_CACHE

    in_maps = make_in_maps(x, gate_perc, alpha, beta)
    res = run_bass_kernel_spmd(nc, in_maps, list(range(NCORES))).results
    return assemble(res)


if __name__ == "__main__":
    rng = np.random.default_rng(0)
    x = rng.standard_normal((B, I)).astype(np.float32)
    g = rng.random((B, E)).astype(np.float32)
    g /= g.sum(-1, keepdims=True)
    al = (rng.standard_normal((E, I, U)) * 0.05).astype(np.float32)
    be = (rng.standard_normal((E, U)) * 0.05).astype(np.float32)
    got = kernel(x, g, al, be)
    ref = np.einsum("bi,eio->beo", x, al, optimize=True)
    ref = np.einsum("beo,be->bo", ref, g) + g @ be
    err = np.abs(got - ref)
    print("max abs err", err.max(), "rel", err.max() / np.abs(ref).max())


# revision 12
# speedup vs baseline: 1.6927x; 1.4867x over previous
"""Trainium2 Bass kernel for nn_DenseExpert (soft-gated mixture of dense experts).

Math:  out[b,u] = sum_e gate[b,e] * (x[b,:] @ alpha[e]) [u] + (gate @ beta)[b,u]

Strategy (pure data parallel over batch, 8 cores). Per 512-row chunk per core:
  1. DMA x/gate chunk (batch-major); cast to fp16 (11-bit mantissa, close to
     TF32 precision; PE streams fp16 at 1 cycle/row; PSUM accumulation fp32).
  2. Build K=64 block-diagonal gate matrices: dstack[p, e, c] =
     gate[p,e]*[c == p%64], one fp16 tensor_tensor per 128-row tile
     (ident-pattern * gate broadcast, FD=512) on DVE.
  3. y_e.T tiles via PE matmuls: for each 64-row block l,
     yT[i, (e, c)] = x[64l:64l+64, :].T @ dstack[64l:64l+64]   (N=512).
     This replaces both a scale stage and per-expert PE transposes.
  4. PSUM->SBUF copies gather yT into [i, e, b] fp16 layout (DVE/ACT split).
  5. PE matmuls accumulate out.T[u,b] = sum_e alpha_e.T @ y_e.T  plus the
     bias as one K=8 matmul beta.T @ gate.T (gate.T via 4 tiny identity
     matmuls).
  6. out.T (fp32) copied to SBUF and DMA'd to DRAM in [U, B] layout; the
     host does the final cheap transpose when assembling the full result.
"""

import dataclasses
from contextlib import ExitStack

import numpy as np

import concourse.bacc as bacc
import concourse.tile as tile
import concourse.mybir as mybir
from concourse.bass_utils import run_bass_kernel_spmd

F32 = mybir.dt.float32
F16 = mybir.dt.float16

B, E, I, U = 65536, 8, 128, 128
NCORES = 8
BLOC = B // NCORES        # 8192 batch rows per core
CHUNK = 512               # batch rows per pipeline chunk
NCHUNK = BLOC // CHUNK    # 16
TPC = CHUNK // 128        # 128-row tiles per chunk
KB = 64                   # contraction block for the diag trick


def _build():
    nc = bacc.Bacc("TRN2", target_bir_lowering=False, debug=False)

    x = nc.dram_tensor("x", [BLOC, I], F32, kind="ExternalInput").ap()
    gate = nc.dram_tensor("gate", [BLOC, E], F32, kind="ExternalInput").ap()
    alpha = nc.dram_tensor("alpha", [E, I, U], F32, kind="ExternalInput").ap()
    beta = nc.dram_tensor("beta", [E, U], F32, kind="ExternalInput").ap()
    ident = nc.dram_tensor("ident", [128, 128], F16, kind="ExternalInput").ap()
    idrep = nc.dram_tensor("idrep", [128, E, KB], F16, kind="ExternalInput").ap()
    # output stays feature-major on HW; host transposes when assembling
    outT = nc.dram_tensor("outT", [U, BLOC], F32, kind="ExternalOutput").ap()

    with tile.TileContext(nc) as tc, ExitStack() as ctx:
        const = ctx.enter_context(tc.tile_pool(name="const", bufs=1))
        xp = ctx.enter_context(tc.tile_pool(name="xp", bufs=6))
        dgp = ctx.enter_context(tc.tile_pool(name="dgp", bufs=8))
        ytp = ctx.enter_context(tc.tile_pool(name="ytp", bufs=4))
        op = ctx.enter_context(tc.tile_pool(name="op", bufs=3))
        ps_yt = ctx.enter_context(tc.tile_pool(name="ps_yt", bufs=3, space="PSUM"))
        ps_ot = ctx.enter_context(tc.tile_pool(name="ps_ot", bufs=1, space="PSUM"))

        # --- constants (cast alpha/beta to fp16 on chip) ---
        alpha_sb = const.tile([128, E, U], F32, tag="alpha")
        nc.sync.dma_start(alpha_sb[:], alpha.rearrange("e i u -> i e u"))
        alpha_h = const.tile([128, E, U], F16, tag="alphah")
        nc.vector.tensor_copy(alpha_h[:], alpha_sb[:])

        ident_h = const.tile([128, 128], F16, tag="identh")
        nc.sync.dma_start(ident_h[:], ident)
        idrep_h = const.tile([128, E, KB], F16, tag="idreph")
        nc.sync.dma_start(idrep_h[:], idrep)

        def emit_front(c):
            row0 = c * CHUNK
            g_sb = xp.tile([128, TPC, E], F32, tag="g")
            nc.sync.dma_start(
                g_sb[:],
                gate[row0 : row0 + CHUNK, :].rearrange("(t p) e -> p t e", p=128),
            )
            # x: SWDGE DMA with fused fp32->fp16 cast (issued from GpSimd,
            # which is otherwise idle)
            x_h = xp.tile([128, TPC, I], F16, tag="xh")
            nc.gpsimd.dma_start(
                x_h[:], x[row0 : row0 + CHUNK, :].rearrange("(t p) i -> p t i", p=128)
            )
            g_h = xp.tile([128, TPC, E], F16, tag="gh")
            nc.vector.tensor_copy(g_h[:], g_sb[:])

            # per 128-row tile: diag build (DVE) + yT matmuls + gather copy
            yT_all = ytp.tile([128, E, TPC, 128], F16, tag="yT")
            for t in range(TPC):
                gview = dataclasses.replace(
                    g_h[:],
                    ap=[[TPC * E, 128], [1, E], [0, KB]],
                    offset=t * E,
                )
                diag = dgp.tile([128, E, KB], F16, tag="diag")
                nc.vector.tensor_tensor(
                    diag[:], idrep_h[:], gview, op=mybir.AluOpType.mult
                )
                yT_ps = ps_yt.tile([128, 2, E, KB], F32, tag="yTps")
                for l in range(2):
                    nc.tensor.matmul(
                        yT_ps[:, l, :, :],
                        x_h[l * KB : (l + 1) * KB, t, :],
                        diag[l * KB : (l + 1) * KB, :, :],
                        start=True,
                        stop=True,
                    )
                dst = dataclasses.replace(
                    yT_all[:],
                    ap=[[E * TPC * 128, 128], [KB, 2], [TPC * 128, E], [1, KB]],
                    offset=t * 128,
                )
                if t == 3:
                    # split the last tile's gather across DVE and ACT
                    dst0 = dataclasses.replace(
                        yT_all[:],
                        ap=[[E * TPC * 128, 128], [TPC * 128, E], [1, KB]],
                        offset=t * 128,
                    )
                    dst1 = dataclasses.replace(
                        yT_all[:],
                        ap=[[E * TPC * 128, 128], [TPC * 128, E], [1, KB]],
                        offset=t * 128 + KB,
                    )
                    nc.vector.tensor_copy(dst0, yT_ps[:, 0, :, :])
                    nc.scalar.copy(dst1, yT_ps[:, 1, :, :])
                else:
                    nc.scalar.copy(dst, yT_ps[:])
            return yT_all

        def emit_back(c, yT_all):
            row0 = c * CHUNK
            oT_ps = ps_ot.tile([128, CHUNK], F32, tag="oTps")
            for e in range(E):
                nc.tensor.matmul(
                    oT_ps[:],
                    alpha_h[:, e, :],
                    yT_all[:, e, :, :],
                    start=(e == 0),
                    stop=(e == E - 1),
                )

            oT_sb = op.tile([128, CHUNK], F32, tag="oT")
            nc.vector.tensor_copy(oT_sb[:, : CHUNK // 2], oT_ps[:, : CHUNK // 2])
            nc.scalar.copy(oT_sb[:, CHUNK // 2 :], oT_ps[:, CHUNK // 2 :])
            nc.sync.dma_start(outT[:, row0 : row0 + CHUNK], oT_sb[:])

        pending = None
        for c in range(NCHUNK):
            front = emit_front(c)
            if pending is not None:
                emit_back(c - 1, pending)
            pending = front
        emit_back(NCHUNK - 1, pending)

    nc.compile()
    return nc


_NC_CACHE = None


def _make_idrep():
    idrep = np.zeros((128, E, KB), np.float16)
    for p in range(128):
        idrep[p, :, p % KB] = 1.0
    return idrep


def make_in_maps(x, gate_perc, alpha, beta):
    x = np.ascontiguousarray(np.asarray(x, dtype=np.float32))
    gate_perc = np.ascontiguousarray(np.asarray(gate_perc, dtype=np.float32))
    alpha = np.ascontiguousarray(np.asarray(alpha, dtype=np.float32))
    beta = np.ascontiguousarray(np.asarray(beta, dtype=np.float32))
    ident = np.eye(128, dtype=np.float16)
    idrep = _make_idrep()
    in_maps = []
    for c in range(NCORES):
        sl = slice(c * BLOC, (c + 1) * BLOC)
        in_maps.append(
            {
                "x": x[sl],
                "gate": gate_perc[sl],
                "alpha": alpha,
                "beta": beta,
                "ident": ident,
                "idrep": idrep,
            }
        )
    return in_maps


def assemble(results, gate_perc, beta):
    # per-core outputs are [U, BLOC] f32; bias gate@beta added on host
    full_T = np.concatenate([results[c]["outT"] for c in range(NCORES)], axis=1)
    out = np.ascontiguousarray(full_T.T)
    out += np.asarray(gate_perc, dtype=np.float32) @ np.asarray(beta, dtype=np.float32)
    return out


def kernel(x, gate_perc, alpha, beta):
    global _NC_CACHE
    if _NC_CACHE is None:
        _NC_CACHE = _build()
    nc = _NC_CACHE

    in_maps = make_in_maps(x, gate_perc, alpha, beta)
    res = run_bass_kernel_spmd(nc, in_maps, list(range(NCORES))).results
    return assemble(res, gate_perc, beta)


if __name__ == "__main__":
    rng = np.random.default_rng(0)
    x = rng.standard_normal((B, I)).astype(np.float32)
    g = rng.random((B, E)).astype(np.float32)
    g /= g.sum(-1, keepdims=True)
    al = (rng.standard_normal((E, I, U)) * 0.05).astype(np.float32)
    be = (rng.standard_normal((E, U)) * 0.05).astype(np.float32)
    got = kernel(x, g, al, be)
    ref = np.einsum("bi,eio->beo", x, al, optimize=True)
    ref = np.einsum("beo,be->bo", ref, g) + g @ be
    err = np.abs(got - ref)
    print("max abs err", err.max(), "rel", err.max() / np.abs(ref).max())


# revision 13
# speedup vs baseline: 1.7846x; 1.0543x over previous
"""Trainium2 Bass kernel for nn_DenseExpert (soft-gated mixture of dense experts).

Math:  out[b,u] = sum_e gate[b,e] * (x[b,:] @ alpha[e]) [u] + (gate @ beta)[b,u]

Strategy (pure data parallel over batch, 8 cores). Per 512-row chunk per core:
  1. DMA x/gate chunk (batch-major); cast to fp16 (11-bit mantissa, close to
     TF32 precision; PE streams fp16 at 1 cycle/row; PSUM accumulation fp32).
  2. Build K=64 block-diagonal gate matrices: dstack[p, e, c] =
     gate[p,e]*[c == p%64], one fp16 tensor_tensor per 128-row tile
     (ident-pattern * gate broadcast, FD=512) on DVE.
  3. y_e.T tiles via PE matmuls: for each 64-row block l,
     yT[i, (e, c)] = x[64l:64l+64, :].T @ dstack[64l:64l+64]   (N=512).
     This replaces both a scale stage and per-expert PE transposes.
  4. PSUM->SBUF copies gather yT into [i, e, b] fp16 layout (DVE/ACT split).
  5. PE matmuls accumulate out.T[u,b] = sum_e alpha_e.T @ y_e.T  plus the
     bias as one K=8 matmul beta.T @ gate.T (gate.T via 4 tiny identity
     matmuls).
  6. out.T (fp32) copied to SBUF and DMA'd to DRAM in [U, B] layout; the
     host does the final cheap transpose when assembling the full result.
"""

import dataclasses
from contextlib import ExitStack

import numpy as np

import concourse.bacc as bacc
import concourse.tile as tile
import concourse.mybir as mybir
from concourse.bass_utils import run_bass_kernel_spmd

F32 = mybir.dt.float32
F16 = mybir.dt.float16

B, E, I, U = 65536, 8, 128, 128
NCORES = 8
BLOC = B // NCORES        # 8192 batch rows per core
CHUNK = 512               # batch rows per pipeline chunk
NCHUNK = BLOC // CHUNK    # 16
TPC = CHUNK // 128        # 128-row tiles per chunk
KB = 64                   # contraction block for the diag trick


def _build():
    nc = bacc.Bacc("TRN2", target_bir_lowering=False, debug=False)

    x = nc.dram_tensor("x", [BLOC, I], F32, kind="ExternalInput").ap()
    gate = nc.dram_tensor("gate", [BLOC, E], F32, kind="ExternalInput").ap()
    # g2[q, c, t, e, j] = gate[512c + 128t + q, e] duplicated over j in {0,1}
    g2 = nc.dram_tensor("g2", [128, NCHUNK, TPC, E, 2], F16, kind="ExternalInput").ap()
    alpha = nc.dram_tensor("alpha", [E, I, U], F32, kind="ExternalInput").ap()
    beta = nc.dram_tensor("beta", [E, U], F32, kind="ExternalInput").ap()
    ident = nc.dram_tensor("ident", [128, 128], F16, kind="ExternalInput").ap()
    idrep = nc.dram_tensor("idrep", [128, E, KB], F16, kind="ExternalInput").ap()
    # output stays feature-major on HW; host transposes when assembling
    outT = nc.dram_tensor("outT", [U, BLOC], F32, kind="ExternalOutput").ap()

    with tile.TileContext(nc) as tc, ExitStack() as ctx:
        const = ctx.enter_context(tc.tile_pool(name="const", bufs=1))
        xp = ctx.enter_context(tc.tile_pool(name="xp", bufs=6))
        dgp = ctx.enter_context(tc.tile_pool(name="dgp", bufs=8))
        ytp = ctx.enter_context(tc.tile_pool(name="ytp", bufs=4))
        op = ctx.enter_context(tc.tile_pool(name="op", bufs=3))
        ps_yt = ctx.enter_context(tc.tile_pool(name="ps_yt", bufs=3, space="PSUM"))
        ps_ot = ctx.enter_context(tc.tile_pool(name="ps_ot", bufs=1, space="PSUM"))

        # --- constants (cast alpha/beta to fp16 on chip) ---
        alpha_sb = const.tile([128, E, U], F32, tag="alpha")
        nc.sync.dma_start(alpha_sb[:], alpha.rearrange("e i u -> i e u"))
        alpha_h = const.tile([128, E, U], F16, tag="alphah")
        nc.vector.tensor_copy(alpha_h[:], alpha_sb[:])

        g2_sb = const.tile([128, NCHUNK, TPC, E, 2], F16, tag="g2")
        nc.sync.dma_start(g2_sb[:], g2)
        g2_pitch = NCHUNK * TPC * E * 2
        ident_h = const.tile([128, 128], F16, tag="identh")
        nc.sync.dma_start(ident_h[:], ident)
        idrep_h = const.tile([128, E, KB], F16, tag="idreph")
        nc.sync.dma_start(idrep_h[:], idrep)

        def emit_front(c):
            row0 = c * CHUNK
            # x: SWDGE DMA with fused fp32->fp16 cast (issued from GpSimd,
            # which is otherwise idle)
            x_h = xp.tile([128, TPC, I], F16, tag="xh")
            nc.gpsimd.dma_start(
                x_h[:], x[row0 : row0 + CHUNK, :].rearrange("(t p) i -> p t i", p=128)
            )

            # per 128-row tile: diag build (DVE) + yT matmuls + gather copy
            yT_all = ytp.tile([128, E, TPC, 128], F16, tag="yT")
            for t in range(TPC):
                diag = dgp.tile([128, E, KB], F16, tag="diag")
                diag_v = dataclasses.replace(
                    diag[:],
                    ap=[[E * KB, 128], [KB, E], [2, KB // 2], [1, 2]],
                    offset=0,
                )
                idrep_v = dataclasses.replace(
                    idrep_h[:],
                    ap=[[E * KB, 128], [KB, E], [2, KB // 2], [1, 2]],
                    offset=0,
                )
                g2_v = dataclasses.replace(
                    g2_sb[:],
                    ap=[[g2_pitch, 128], [2, E], [0, KB // 2], [1, 2]],
                    offset=(c * TPC + t) * E * 2,
                )
                nc.vector.tensor_tensor(
                    diag_v, idrep_v, g2_v, op=mybir.AluOpType.mult
                )
                yT_ps = ps_yt.tile([128, 2, E, KB], F32, tag="yTps")
                for l in range(2):
                    nc.tensor.matmul(
                        yT_ps[:, l, :, :],
                        x_h[l * KB : (l + 1) * KB, t, :],
                        diag[l * KB : (l + 1) * KB, :, :],
                        start=True,
                        stop=True,
                    )
                dst = dataclasses.replace(
                    yT_all[:],
                    ap=[[E * TPC * 128, 128], [KB, 2], [TPC * 128, E], [1, KB]],
                    offset=t * 128,
                )
                if t == 3:
                    # split the last tile's gather across DVE and ACT
                    dst0 = dataclasses.replace(
                        yT_all[:],
                        ap=[[E * TPC * 128, 128], [TPC * 128, E], [1, KB]],
                        offset=t * 128,
                    )
                    dst1 = dataclasses.replace(
                        yT_all[:],
                        ap=[[E * TPC * 128, 128], [TPC * 128, E], [1, KB]],
                        offset=t * 128 + KB,
                    )
                    nc.vector.tensor_copy(dst0, yT_ps[:, 0, :, :])
                    nc.scalar.copy(dst1, yT_ps[:, 1, :, :])
                else:
                    nc.scalar.copy(dst, yT_ps[:])
            return yT_all

        def emit_back(c, yT_all):
            row0 = c * CHUNK
            oT_ps = ps_ot.tile([128, CHUNK], F32, tag="oTps")
            for e in range(E):
                nc.tensor.matmul(
                    oT_ps[:],
                    alpha_h[:, e, :],
                    yT_all[:, e, :, :],
                    start=(e == 0),
                    stop=(e == E - 1),
                )

            oT_sb = op.tile([128, CHUNK], F32, tag="oT")
            nc.vector.tensor_copy(oT_sb[:, : CHUNK // 2], oT_ps[:, : CHUNK // 2])
            nc.scalar.copy(oT_sb[:, CHUNK // 2 :], oT_ps[:, CHUNK // 2 :])
            nc.sync.dma_start(outT[:, row0 : row0 + CHUNK], oT_sb[:])

        pending = None
        for c in range(NCHUNK):
            front = emit_front(c)
            if pending is not None:
                emit_back(c - 1, pending)
            pending = front
        emit_back(NCHUNK - 1, pending)

    nc.compile()
    return nc


_NC_CACHE = None


def _make_idrep():
    idrep = np.zeros((128, E, KB), np.float16)
    for p in range(128):
        idrep[p, :, p % KB] = 1.0
    return idrep


def make_in_maps(x, gate_perc, alpha, beta):
    x = np.ascontiguousarray(np.asarray(x, dtype=np.float32))
    gate_perc = np.ascontiguousarray(np.asarray(gate_perc, dtype=np.float32))
    alpha = np.ascontiguousarray(np.asarray(alpha, dtype=np.float32))
    beta = np.ascontiguousarray(np.asarray(beta, dtype=np.float32))
    ident = np.eye(128, dtype=np.float16)
    idrep = _make_idrep()
    in_maps = []
    for c in range(NCORES):
        sl = slice(c * BLOC, (c + 1) * BLOC)
        # g2[q, ch, t, e, j] = gate[c*BLOC + 512ch + 128t + q, e], j duplicated
        gc = gate_perc[sl].astype(np.float16).reshape(NCHUNK, TPC, 128, E)
        gc = gc.transpose(2, 0, 1, 3)  # [q, ch, t, e]
        g2 = np.ascontiguousarray(
            np.broadcast_to(gc[..., None], (128, NCHUNK, TPC, E, 2))
        )
        in_maps.append(
            {
                "x": x[sl],
                "gate": gate_perc[sl],
                "alpha": alpha,
                "beta": beta,
                "ident": ident,
                "idrep": idrep,
                "g2": g2,
            }
        )
    return in_maps


def assemble(results, gate_perc, beta):
    # per-core outputs are [U, BLOC] f32; bias gate@beta added on host
    full_T = np.concatenate([results[c]["outT"] for c in range(NCORES)], axis=1)
    out = np.ascontiguousarray(full_T.T)
    out += np.asarray(gate_perc, dtype=np.float32) @ np.asarray(beta, dtype=np.float32)
    return out


def kernel(x, gate_perc, alpha, beta):
    global _NC_CACHE
    if _NC_CACHE is None:
        _NC_CACHE = _build()
    nc = _NC_CACHE

    in_maps = make_in_maps(x, gate_perc, alpha, beta)
    res = run_bass_kernel_spmd(nc, in_maps, list(range(NCORES))).results
    return assemble(res, gate_perc, beta)


if __name__ == "__main__":
    rng = np.random.default_rng(0)
    x = rng.standard_normal((B, I)).astype(np.float32)
    g = rng.random((B, E)).astype(np.float32)
    g /= g.sum(-1, keepdims=True)
    al = (rng.standard_normal((E, I, U)) * 0.05).astype(np.float32)
    be = (rng.standard_normal((E, U)) * 0.05).astype(np.float32)
    got = kernel(x, g, al, be)
    ref = np.einsum("bi,eio->beo", x, al, optimize=True)
    ref = np.einsum("beo,be->bo", ref, g) + g @ be
    err = np.abs(got - ref)
    print("max abs err", err.max(), "rel", err.max() / np.abs(ref).max())
